# revision 1
# baseline (speedup 1.0000x reference)
"""DG-block (dual graph-conv) Trainium2 kernel — nn_DG_Block.

Reference per batch item b (B=8, C=128, N=2000, K=9):
  idx1 = top9(knn keys on features_b); idx2 = top9(... motion_b)
  gf_i = graph_feature(features_b, idx_i) -> [2C, N, 9]
  f_i  = conv_bn_relu(1x3 stride 3) -> conv_bn_relu(1x3) on gf_i
  out_b = f1 + delta * f2        [C, N, 1]
BatchNorm pools over the WHOLE batch -> stats are all-reduced across cores.

Sharding: one batch item per NeuronCore (8 cores); conv/BN params replicated;
two [128,4] AllReduces reproduce the exact batch statistics.

Algebra (per branch; w1 [C,2C,1,3] split A_d = w1[:,:C,0,d], B_d = w1[:,C:,0,d]):
  conv1[o,n,t] = (P x_n)[o] - sum_d (B_d x_{idx[n,3t+d]})[o],  P = sum_d A_d+B_d
  (conv biases dropped: BN mean-subtraction cancels them exactly)
  knn rank key: <x_i, x_j> - |x_j|^2/2  (monotone per-row transform of the
  reference's 2<x_i,x_j> - |x_i|^2 - |x_j|^2; rank-1 is always the point
  itself -> hardcoded; ranks 2..9 via DVE max8 + max_index with the diagonal
  masked to -1e30)

Device pipeline per core:
  kNN    : pd chunk [128,2048] = X_chunk^T X on PE (fp32) + (-|x_j|^2/2) via
           SWDGE broadcast-accumulate DMA; max8 + max_index on DVE.
  tables : ytab[n, d*C:..] = (-B_d x_n)^T and Z^T rows (PE, ACT copy, DMA).
  conv1  : per (chunk,t): g = Z^T rows + 3 gather-accumulate taps from ytab
           (indirect SWDGE, compute_op=add), PE-transpose -> o1 psum;
           ACT Copy/Square with accum_out -> stats; AllReduce; ACT Relu-affine.
  conv2  : 3 accumulated matmuls; stats; AllReduce; final Relu-affines,
           f1 + delta*f2 on DVE, DMA out.
"""

import numpy as np

import concourse.bacc as bacc
import concourse.bass as bass
import concourse.mybir as mybir
import concourse.tile as tile
import concourse.bass_utils as bass_utils
from concourse.masks import make_identity

F32 = mybir.dt.float32
U32 = mybir.dt.uint32
AF = mybir.ActivationFunctionType
ALU = mybir.AluOpType

B = 8
C = 128
N = 2000
EPS = 1e-5
NEG_BIG = -1.0e30

CHUNKS = [(i * 128, min(128, N - i * 128)) for i in range((N + 127) // 128)]
NCH = len(CHUNKS)  # 16
# pd column tiles, 512-aligned so the diagonal block never straddles tiles
JT = [(j * 512, min(512, N - j * 512)) for j in range(4)]


def build_kernel(delta_nonneg: bool):
    nc = bacc.Bacc(
        "TRN2",
        target_bir_lowering=False,
        debug=False,
        enable_asserts=False,
        num_devices=B,
    )

    feat_in = nc.dram_tensor("feat", [C, N], F32, kind="ExternalInput").ap()
    mot_in = nc.dram_tensor("mot", [C, N], F32, kind="ExternalInput").ap()
    wb = {}
    for br in (1, 2):
        wb[br] = {
            "pt": nc.dram_tensor(f"pt{br}", [C, C], F32, kind="ExternalInput").ap(),
            "nbt": nc.dram_tensor(f"nbt{br}", [C, 3 * C], F32, kind="ExternalInput").ap(),
            "w2t": nc.dram_tensor(f"w2t{br}", [C, 3 * C], F32, kind="ExternalInput").ap(),
            "bn": nc.dram_tensor(f"bn{br}", [C, 4], F32, kind="ExternalInput").ap(),
        }
    delta_in = nc.dram_tensor("delta", [1, 1], F32, kind="ExternalInput").ap()
    out_t = nc.dram_tensor("out", [C, N], F32, kind="ExternalOutput").ap()

    with tile.TileContext(nc) as tc:
        _emit(nc, tc, feat_in, mot_in, wb, delta_in, out_t, delta_nonneg)
    nc.compile()
    return nc


def _emit(nc, tc, feat_in, mot_in, wb, delta_in, out_t, delta_nonneg):
    import contextlib

    ctx = contextlib.ExitStack()
    with ctx:
        sb = ctx.enter_context(tc.tile_pool(name="sb", bufs=1))
        pd_ps = ctx.enter_context(tc.tile_pool(name="pd_ps", bufs=2, space="PSUM"))
        st_ps = ctx.enter_context(tc.tile_pool(name="st_ps", bufs=2, space="PSUM"))
        o1_ps = ctx.enter_context(tc.tile_pool(name="o1_ps", bufs=2, space="PSUM"))
        dr = ctx.enter_context(tc.tile_pool(name="dr", bufs=1, space="DRAM"))

        # ---------------- persistent on-chip data ----------------
        x = sb.tile([C, N], F32, name="x")
        nc.sync.dma_start(out=x[:], in_=feat_in)
        m = sb.tile([C, N], F32, name="m")
        nc.sync.dma_start(out=m[:], in_=mot_in)

        ident = sb.tile([C, C], F32, name="ident")
        make_identity(nc, ident[:])
        ineg = sb.tile([C, C], F32, name="ineg")
        nc.scalar.activation(out=ineg[:], in_=ident[:], func=AF.Copy, scale=NEG_BIG)
        ones1 = sb.tile([1, C], F32, name="ones1")
        nc.vector.memset(ones1[:], 1.0)
        neghalfc = sb.tile([C, 1], F32, name="neghalfc")
        nc.vector.memset(neghalfc[:], -0.5)

        w = {}
        for br in (1, 2):
            pt = sb.tile([C, C], F32, name=f"pt{br}")
            nc.sync.dma_start(out=pt[:], in_=wb[br]["pt"])
            nbt = sb.tile([C, 3 * C], F32, name=f"nbt{br}")
            nc.sync.dma_start(out=nbt[:], in_=wb[br]["nbt"])
            w2t = sb.tile([C, 3 * C], F32, name=f"w2t{br}")
            nc.sync.dma_start(out=w2t[:], in_=wb[br]["w2t"])
            bn = sb.tile([C, 4], F32, name=f"bn{br}")
            nc.sync.dma_start(out=bn[:], in_=wb[br]["bn"])
            w[br] = dict(pt=pt, nbt=nbt, w2t=w2t, bn=bn)

        delta_sb = sb.tile([1, 1], F32, name="delta_sb")
        nc.sync.dma_start(out=delta_sb[:], in_=delta_in)
        # broadcast delta to a [C,1] column via K=1 matmul
        dps = st_ps.tile([C, 8], F32, name="dps", tag="stage")
        nc.tensor.matmul(
            out=dps[:, 0:1], lhsT=ones1[:], rhs=delta_sb[0:1, 0:1], start=True, stop=True
        )
        dcol = sb.tile([C, 1], F32, name="dcol")
        nc.scalar.activation(out=dcol[:], in_=dps[:, 0:1], func=AF.Copy)

        ytab = {br: dr.tile([N, 3 * C], F32, name=f"ytab{br}") for br in (1, 2)}
        idx8 = {s: sb.tile([C, NCH * 8], U32, name=f"idx8_{s}") for s in (1, 2)}

        # ---------------- kNN ----------------
        def knn(src, which):
            # xsq = src*src (ACT); sqrow = -0.5 * colsum(xsq) (PE); -> DRAM
            xsq = sb.tile([C, N], F32, name=f"xsq_{which}", tag="xsq", bufs=1)
            nc.scalar.activation(out=xsq[:], in_=src[:], func=AF.Square)
            sqrow = sb.tile([1, N], F32, name=f"sqrow_{which}", tag="sqrow", bufs=1)
            for j0, jn in JT:
                sqps = st_ps.tile([1, 512], F32, name=f"sqps_{which}_{j0}", tag="stage")
                nc.tensor.matmul(
                    out=sqps[0:1, :jn],
                    lhsT=neghalfc[:],
                    rhs=xsq[:, j0 : j0 + jn],
                    start=True,
                    stop=True,
                )
                nc.scalar.activation(
                    out=sqrow[0:1, j0 : j0 + jn], in_=sqps[0:1, :jn], func=AF.Copy
                )
            negsqh = sb.tile([C, N], F32, name=f"negsqh_{which}", tag="negsqh", bufs=1)
            for j0, jn in JT:
                nps = st_ps.tile([C, 512], F32, name=f"nps_{which}_{j0}", tag="stage")
                nc.tensor.matmul(
                    out=nps[:, :jn],
                    lhsT=ones1[:],
                    rhs=sqrow[0:1, j0 : j0 + jn],
                    start=True,
                    stop=True,
                )
                nc.scalar.activation(
                    out=negsqh[:, j0 : j0 + jn], in_=nps[:, :jn], func=AF.Copy
                )

            for ci, (c0, cn) in enumerate(CHUNKS):
                pdt = sb.tile([C, N], F32, name=f"pd_{which}_{ci}", tag="pd", bufs=2)
                for half in range(2):
                    pps = pd_ps.tile(
                        [C, 1024], F32, name=f"pps_{which}_{ci}_{half}", tag="pdps"
                    )
                    for sub in range(2):
                        j0, jn = JT[half * 2 + sub]
                        nc.tensor.matmul(
                            out=pps[:cn, sub * 512 : sub * 512 + jn],
                            lhsT=src[:, c0 : c0 + cn],
                            rhs=src[:, j0 : j0 + jn],
                            start=True,
                            stop=True,
                        )
                    w0 = JT[half * 2][1] + JT[half * 2 + 1][1]
                    nc.scalar.activation(
                        out=pdt[:cn, half * 1024 : half * 1024 + w0],
                        in_=pps[:cn, 0:w0],
                        func=AF.Copy,
                    )
                # accumulate -|x_j|^2/2 broadcast row (DVE; GpSimd is the
                # kernel bottleneck, Vector has slack)
                nc.vector.tensor_tensor(
                    out=pdt[:cn, :],
                    in0=pdt[:cn, :],
                    in1=negsqh[:cn, :],
                    op=ALU.add,
                )
                # mask the diagonal block on DVE (adds commute)
                nc.vector.tensor_tensor(
                    out=pdt[:cn, c0 : c0 + cn],
                    in0=pdt[:cn, c0 : c0 + cn],
                    in1=ineg[:cn, :cn],
                    op=ALU.add,
                )
                vals8 = sb.tile([C, 8], F32, name=f"v8_{which}_{ci}", tag="v8", bufs=2)
                nc.vector.max(out=vals8[:cn], in_=pdt[:cn, :])
                nc.vector.max_index(
                    out=idx8[which][:cn, ci * 8 : ci * 8 + 8],
                    in_max=vals8[:cn],
                    in_values=pdt[:cn, :],
                )

        # ---------------- Y/Z tables ----------------
        ztiles = {}

        def tables(br):
            zl = []
            for ci, (c0, cn) in enumerate(CHUNKS):
                yps = st_ps.tile([C, 384], F32, name=f"yps_{br}_{ci}", tag="stage")
                nc.tensor.matmul(
                    out=yps[:cn, :],
                    lhsT=x[:, c0 : c0 + cn],
                    rhs=w[br]["nbt"][:],
                    start=True,
                    stop=True,
                )
                yst = sb.tile([C, 384], F32, name=f"yst_{br}_{ci}", tag="yst", bufs=3)
                nc.scalar.activation(out=yst[:cn, :], in_=yps[:cn, :], func=AF.Copy)
                nc.sync.dma_start(out=ytab[br][c0 : c0 + cn, :], in_=yst[:cn, :])

                zps = st_ps.tile([C, 128], F32, name=f"zps_{br}_{ci}", tag="stage")
                nc.tensor.matmul(
                    out=zps[:cn, :],
                    lhsT=x[:, c0 : c0 + cn],
                    rhs=w[br]["pt"][:],
                    start=True,
                    stop=True,
                )
                zt = sb.tile([C, C], F32, name=f"zt_{br}_{ci}", tag=f"zt{br}", bufs=NCH)
                nc.scalar.activation(out=zt[:cn, :], in_=zps[:cn, :], func=AF.Copy)
                zl.append(zt)
            ztiles[br] = zl

        # ---------------- conv1 + stats ----------------
        o1_tiles = {}
        stats1 = {}

        def conv1(br, which):
            ol = []
            s1 = sb.tile([C, NCH], F32, name=f"s1c_{br}")
            s2 = sb.tile([C, NCH], F32, name=f"s2c_{br}")
            for ci, (c0, cn) in enumerate(CHUNKS):
                ops = o1_ps.tile([C, 384], F32, name=f"o1ps_{br}_{ci}", tag="o1")
                for t in range(3):
                    g = sb.tile([C, C], F32, name=f"g_{br}_{ci}_{t}", tag="g", bufs=9)
                    nc.sync.dma_start(out=g[:cn, :], in_=ztiles[br][ci][:cn, :])
                    g0 = None
                    for d in range(3):
                        j = 3 * t + d
                        if j == 0:
                            # self tap: contiguous rows -> HWDGE into its own
                            # tile, summed by an extra accumulated transpose
                            g0 = sb.tile(
                                [C, C], F32, name=f"g0_{br}_{ci}", tag="g0", bufs=3
                            )
                            nc.sync.dma_start(
                                out=g0[:cn, :], in_=ytab[br][c0 : c0 + cn, 0:C]
                            )
                        else:
                            nc.gpsimd.indirect_dma_start(
                                out=g[:cn, :],
                                out_offset=None,
                                in_=ytab[br][:, :],
                                in_offset=bass.IndirectOffsetOnAxis(
                                    ap=idx8[which][:cn, ci * 8 + j - 1 : ci * 8 + j],
                                    axis=0,
                                ),
                                element_offset=d * C,
                                compute_op=ALU.add,
                            )
                    nc.tensor.matmul(
                        out=ops[:, t * C : t * C + cn],
                        lhsT=g[:cn, :],
                        rhs=ident[:cn, :cn],
                        is_transpose=True,
                        start=True,
                        stop=(g0 is None),
                        skip_group_check=True,
                    )
                    if g0 is not None:
                        nc.tensor.matmul(
                            out=ops[:, t * C : t * C + cn],
                            lhsT=g0[:cn, :],
                            rhs=ident[:cn, :cn],
                            is_transpose=True,
                            start=False,
                            stop=True,
                            skip_group_check=True,
                        )
                # stats + store o1 (strided [C,3,cn] views skip padding columns)
                src_ap = ops[:, 0:384].rearrange("p (t n) -> p t n", t=3)[:, :, :cn]
                ot = sb.tile([C, 384], F32, name=f"o1_{br}_{ci}", tag=f"o1{br}", bufs=NCH)
                dst_ap = ot[:, 0:384].rearrange("p (t n) -> p t n", t=3)[:, :, :cn]
                nc.scalar.activation(
                    out=dst_ap, in_=src_ap, func=AF.Copy, accum_out=s1[:, ci : ci + 1]
                )
                osq = sb.tile([C, 384], F32, name=f"o1sq_{br}_{ci}", tag="o1sq", bufs=2)
                sq_ap = osq[:, 0:384].rearrange("p (t n) -> p t n", t=3)[:, :, :cn]
                nc.scalar.activation(
                    out=sq_ap, in_=src_ap, func=AF.Square, accum_out=s2[:, ci : ci + 1]
                )
                ol.append(ot)
            o1_tiles[br] = ol
            stats1[br] = (s1, s2)

        # ---------------- allreduce + affine computation ----------------
        def allreduce_affine(stats_br, m_count, bn_cols, round_id, br):
            """single-branch AllReduce of (sum, sumsq) -> (a_col, c_col)."""
            s1, s2 = stats_br
            arq = sb.tile([C, 2], F32, name=f"arq{round_id}")
            nc.vector.reduce_sum(
                out=arq[:, 0:1], in_=s1[:], axis=mybir.AxisListType.X
            )
            nc.vector.reduce_sum(
                out=arq[:, 1:2], in_=s2[:], axis=mybir.AxisListType.X
            )
            ar_in = dr.tile([C, 2], F32, name=f"arin{round_id}")
            ar_out = dr.tile([C, 2], F32, name=f"arout{round_id}", addr_space="Shared")
            nc.sync.dma_start(out=ar_in[:], in_=arq[:])
            nc.gpsimd.collective_compute(
                "AllReduce",
                ALU.add,
                replica_groups=[list(range(B))],
                ins=[ar_in[:].opt()],
                outs=[ar_out[:].opt()],
            )
            art = sb.tile([C, 2], F32, name=f"art{round_id}")
            nc.sync.dma_start(out=art[:], in_=ar_out[:])

            inv_m = 1.0 / float(m_count)
            if True:
                i = 0
                gcol = w[br]["bn"][:, bn_cols[0] : bn_cols[0] + 1]
                bcol = w[br]["bn"][:, bn_cols[1] : bn_cols[1] + 1]
                mean = sb.tile([C, 1], F32, name=f"mean{round_id}_{br}")
                nc.vector.tensor_scalar_mul(mean[:], art[:, 2 * i : 2 * i + 1], inv_m)
                ey2 = sb.tile([C, 1], F32, name=f"ey2{round_id}_{br}")
                nc.vector.tensor_scalar_mul(ey2[:], art[:, 2 * i + 1 : 2 * i + 2], inv_m)
                var = sb.tile([C, 1], F32, name=f"var{round_id}_{br}")
                nc.vector.tensor_tensor(out=var[:], in0=mean[:], in1=mean[:], op=ALU.mult)
                nc.vector.tensor_tensor(out=var[:], in0=ey2[:], in1=var[:], op=ALU.subtract)
                nc.vector.tensor_scalar_add(var[:], var[:], EPS)
                rv = sb.tile([C, 1], F32, name=f"rv{round_id}_{br}")
                nc.vector.reciprocal(rv[:], var[:])
                rstd = sb.tile([C, 1], F32, name=f"rstd{round_id}_{br}")
                nc.scalar.activation(out=rstd[:], in_=rv[:], func=AF.Sqrt)
                a_col = sb.tile([C, 1], F32, name=f"acol{round_id}_{br}")
                nc.vector.tensor_tensor(out=a_col[:], in0=gcol, in1=rstd[:], op=ALU.mult)
                c_col = sb.tile([C, 1], F32, name=f"ccol{round_id}_{br}")
                nc.vector.tensor_tensor(out=c_col[:], in0=mean[:], in1=a_col[:], op=ALU.mult)
                nc.vector.tensor_tensor(out=c_col[:], in0=bcol, in1=c_col[:], op=ALU.subtract)
            return (a_col, c_col)

        # ---------------- conv2 + stats ----------------
        o2_tiles = {}
        stats2 = {}

        def conv2(br, aff):
            a_col, c_col = aff
            ol = []
            s1 = sb.tile([C, NCH], F32, name=f"s1d_{br}")
            s2 = sb.tile([C, NCH], F32, name=f"s2d_{br}")
            for ci, (c0, cn) in enumerate(CHUNKS):
                ot = o1_tiles[br][ci]
                o1r_ap = ot[:, 0:384].rearrange("p (t n) -> p t n", t=3)[:, :, :cn]
                nc.scalar.activation(
                    out=o1r_ap, in_=o1r_ap, func=AF.Relu, scale=a_col[:], bias=c_col[:]
                )
                o2ps = st_ps.tile([C, 128], F32, name=f"o2ps_{br}_{ci}", tag="stage")
                for d in range(3):
                    nc.tensor.matmul(
                        out=o2ps[:, :cn],
                        lhsT=w[br]["w2t"][:, d * C : (d + 1) * C],
                        rhs=ot[:, d * C : d * C + cn],
                        start=(d == 0),
                        stop=(d == 2),
                    )
                o2 = sb.tile([C, C], F32, name=f"o2_{br}_{ci}", tag=f"o2{br}", bufs=NCH)
                nc.scalar.activation(
                    out=o2[:, :cn],
                    in_=o2ps[:, :cn],
                    func=AF.Copy,
                    accum_out=s1[:, ci : ci + 1],
                )
                osq = sb.tile([C, C], F32, name=f"o2sq_{br}_{ci}", tag="o2sq", bufs=2)
                nc.scalar.activation(
                    out=osq[:, :cn],
                    in_=o2ps[:, :cn],
                    func=AF.Square,
                    accum_out=s2[:, ci : ci + 1],
                )
                ol.append(o2)
            o2_tiles[br] = ol
            stats2[br] = (s1, s2)

        # ---------------- emit ----------------
        knn(x, 1)
        tables(1)
        knn(m, 2)
        conv1(1, 1)
        tables(2)
        conv1(2, 2)

        # branch-1 post-kNN chain is emitted first so its ARs/conv2/finals
        # overlap branch-2's gather wall (GpSimd is the bottleneck there)
        aff1_1 = allreduce_affine(stats1[1], B * N * 3, (0, 1), "1a", 1)
        conv2(1, aff1_1)
        aff2_1 = allreduce_affine(stats2[1], B * N, (2, 3), "2a", 1)
        a1, c1 = aff2_1
        f1_tiles = []
        for ci, (c0, cn) in enumerate(CHUNKS):
            f1t = sb.tile([C, C], F32, name=f"f1_{ci}", tag="f1", bufs=NCH)
            nc.scalar.activation(
                out=f1t[:, :cn],
                in_=o2_tiles[1][ci][:, :cn],
                func=AF.Relu,
                scale=a1[:],
                bias=c1[:],
            )
            f1_tiles.append(f1t)

        aff1_2 = allreduce_affine(stats1[2], B * N * 3, (0, 1), "1b", 2)
        conv2(2, aff1_2)
        aff2_2 = allreduce_affine(stats2[2], B * N, (2, 3), "2b", 2)
        # fold delta into branch-2 affine when delta >= 0
        a2, c2 = aff2_2
        if delta_nonneg:
            a2d = sb.tile([C, 1], F32, name="a2d")
            nc.vector.tensor_tensor(out=a2d[:], in0=a2[:], in1=dcol[:], op=ALU.mult)
            c2d = sb.tile([C, 1], F32, name="c2d")
            nc.vector.tensor_tensor(out=c2d[:], in0=c2[:], in1=dcol[:], op=ALU.mult)
        for ci, (c0, cn) in enumerate(CHUNKS):
            f1t = f1_tiles[ci]
            f2t = sb.tile([C, C], F32, name=f"f2_{ci}", tag="f2", bufs=2)
            if delta_nonneg:
                nc.scalar.activation(
                    out=f2t[:, :cn],
                    in_=o2_tiles[2][ci][:, :cn],
                    func=AF.Relu,
                    scale=a2d[:],
                    bias=c2d[:],
                )
            else:
                nc.scalar.activation(
                    out=f2t[:, :cn],
                    in_=o2_tiles[2][ci][:, :cn],
                    func=AF.Relu,
                    scale=a2[:],
                    bias=c2[:],
                )
                nc.vector.tensor_scalar_mul(f2t[:, :cn], f2t[:, :cn], dcol[:])
            of = sb.tile([C, C], F32, name=f"of_{ci}", tag="of", bufs=2)
            nc.vector.tensor_tensor(
                out=of[:, :cn], in0=f1t[:, :cn], in1=f2t[:, :cn], op=ALU.add
            )
            nc.sync.dma_start(out=out_t[:, c0 : c0 + cn], in_=of[:, :cn])


# ======================= host side =======================

_CACHE = {}


def _prep_branch(w1, b1, g1, be1, w2, b2, g2, be2):
    w1 = np.asarray(w1, dtype=np.float32)
    w2 = np.asarray(w2, dtype=np.float32)
    A = w1[:, :C, 0, :]  # [o, i, 3]
    Bm = w1[:, C:, 0, :]  # [o, i, 3]
    pt = np.ascontiguousarray((A + Bm).sum(axis=2).T)  # [i, o]
    nbt = np.ascontiguousarray(
        np.concatenate([(-Bm[:, :, d]).T for d in range(3)], axis=1)
    )  # [i, 3C]
    w2t = np.ascontiguousarray(
        np.concatenate([w2[:, :, 0, d].T for d in range(3)], axis=1)
    )  # [o, 3C]
    bn = np.ascontiguousarray(
        np.stack(
            [
                np.asarray(g1, np.float32),
                np.asarray(be1, np.float32),
                np.asarray(g2, np.float32),
                np.asarray(be2, np.float32),
            ],
            axis=1,
        )
    )  # [C, 4]
    return pt, nbt, w2t, bn


def kernel(**inputs):
    features = np.ascontiguousarray(np.asarray(inputs["features"], np.float32))
    motion = np.ascontiguousarray(np.asarray(inputs["motion"], np.float32))
    delta = np.asarray(inputs["delta"], np.float32).reshape(-1)[0]

    pt1, nbt1, w2t1, bn1 = _prep_branch(
        inputs["d1_w1"], inputs["d1_b1"], inputs["d1_g1"], inputs["d1_be1"],
        inputs["d1_w2"], inputs["d1_b2"], inputs["d1_g2"], inputs["d1_be2"],
    )
    pt2, nbt2, w2t2, bn2 = _prep_branch(
        inputs["d2_w1"], inputs["d2_b1"], inputs["d2_g1"], inputs["d2_be1"],
        inputs["d2_w2"], inputs["d2_b2"], inputs["d2_g2"], inputs["d2_be2"],
    )

    delta_nonneg = bool(delta >= 0.0)
    key = ("dg", delta_nonneg)
    if key not in _CACHE:
        _CACHE[key] = build_kernel(delta_nonneg)
    nc = _CACHE[key]

    shared = {
        "pt1": pt1, "nbt1": nbt1, "w2t1": w2t1, "bn1": bn1,
        "pt2": pt2, "nbt2": nbt2, "w2t2": w2t2, "bn2": bn2,
        "delta": np.array([[delta]], np.float32),
    }
    in_maps = []
    for c in range(B):
        im = dict(shared)
        im["feat"] = np.ascontiguousarray(features[c, :, :, 0])
        im["mot"] = np.ascontiguousarray(motion[c, :, :, 0])
        in_maps.append(im)

    import os

    trace = bool(int(os.environ.get("DG_KERNEL_TRACE", "0")))
    res = bass_utils.run_bass_kernel_spmd(
        nc, in_maps, core_ids=list(range(B)), trace=trace
    )
    global LAST_RESULTS
    LAST_RESULTS = res
    out = np.stack([res.results[c]["out"] for c in range(B)], axis=0)
    return out.reshape(B, C, N, 1).astype(np.float32)


LAST_RESULTS = None

